# revision 13
# baseline (speedup 1.0000x reference)
import numpy as np

B = 128
FEAT = 64
LATENT = 512
OUT_F = 6144  # NUM_POINTS * 3
EPS = 1e-5
N_CORES = 8
SEGS_PER_CORE = 16
S_PAD = 8192
F16MIN = np.float16(-65504.0)

_CACHE = {}


def build_nc():
    from concourse import bass, bacc, tile

    mybir = bass.mybir
    f32 = mybir.dt.float32
    f16 = mybir.dt.float16
    bf16 = mybir.dt.bfloat16
    AF = mybir.ActivationFunctionType
    ALU = mybir.AluOpType
    X = mybir.AxisListType.X

    nc = bacc.Bacc("TRN2")
    xt_d = nc.declare_dram_parameter("xt", [128, 8, S_PAD], f16, isOutput=False)
    wp_d = nc.declare_dram_parameter("wp2", [128, LATENT], bf16, isOutput=False)
    bp_d = nc.declare_dram_parameter("bp", [128, 4], f32, isOutput=False)
    w1_d = nc.declare_dram_parameter("w1p", [128, 1024], bf16, isOutput=False)
    b1_d = nc.declare_dram_parameter("b1p", [128, 2], f32, isOutput=False)
    w2_d = nc.declare_dram_parameter("w2p", [128, 1024], bf16, isOutput=False)
    b2_d = nc.declare_dram_parameter("b2p", [128, 4], f32, isOutput=False)
    w3_d = nc.declare_dram_parameter("w3p", [128, 3, 4, 2048], bf16, isOutput=False)
    selT_d = nc.declare_dram_parameter("selT", [128, 2], f32, isOutput=False)
    sel_d = nc.declare_dram_parameter("sel", [2, 128], f32, isOutput=False)
    # col-tiled GEMM output: row 32*sub+s, col 512*g+n -> out[16c+s, 2048g+512sub+n]
    out_d = nc.declare_dram_parameter("out", [128, 1536], f16, isOutput=True)

    with tile.TileContext(nc) as tc:
        with (
            tc.tile_pool(name="wpool", bufs=1) as wpool,
            tc.tile_pool(name="fpool", bufs=6) as fpool,
            tc.tile_pool(name="spool", bufs=1) as spool,
            tc.tile_pool(name="ps_s", bufs=1, space=bass.MemorySpace.PSUM) as ps_s,
            tc.tile_pool(name="ps_b", bufs=1, space=bass.MemorySpace.PSUM) as ps_b,
            tc.tile_pool(name="ps_m", bufs=3, space=bass.MemorySpace.PSUM) as ps_m,
            tc.tile_pool(name="ps_o", bufs=2, space=bass.MemorySpace.PSUM) as ps_o,
        ):
            wp = wpool.tile([128, LATENT], bf16)
            bp = wpool.tile([128, 4], f32)
            w1 = wpool.tile([128, 1024], bf16)
            b1 = wpool.tile([128, 2], f32)
            w2 = wpool.tile([128, 1024], bf16)
            b2 = wpool.tile([128, 4], f32)
            sel2T = wpool.tile([128, 2], f32)
            sel2 = wpool.tile([2, 128], f32)
            eps_t = wpool.tile([2, 1], f32)
            scr = wpool.tile([2, 1], f32)
            scr2 = wpool.tile([128, 3], f16)
            w3t = wpool.tile([128, 3, 4, 2048], bf16)

            nc.gpsimd.dma_start(sel2T[:], selT_d[:])
            nc.gpsimd.dma_start(sel2[:], sel_d[:])
            for t, d in (
                (wp, wp_d), (bp, bp_d), (w1, w1_d), (b1, b1_d),
                (w2, w2_d), (b2, b2_d),
            ):
                nc.gpsimd.dma_start(t[:], d[:])

            nc.vector.memset(eps_t[:], EPS)
            # hoist the Sqrt activation-table load out of the tail
            nc.scalar.activation(scr[:], eps_t[:], AF.Sqrt)

            val_h = spool.tile([128, 8], f16)
            val32 = spool.tile([128, 8], f32)
            mu2 = spool.tile([2, 8], f32)
            std = spool.tile([2, 8], f32)
            rstd = spool.tile([2, 8], f32)
            zc = spool.tile([128, 8], f32)
            zsq = spool.tile([128, 8], f32)
            zn16 = spool.tile([128, 16], bf16)
            lat = spool.tile([128, 64], bf16)
            h1 = spool.tile([128, 32], bf16)
            h2 = spool.tile([128, 64], bf16)
            out_sb = spool.tile([128, 1536], f16)
            nc.vector.memset(zn16[:], 0.0)

            # --- segment max pooling. Per tile [128, 8, 1024]: 7 in-place
            # tensor_tensor max folds (2-byte TT runs 2 elem/cyc on DVE vs
            # reduce's 1) then a short 1024-col reduce. ---
            def fold_reduce(ft, t):
                for j in range(1, 8):
                    nc.vector.tensor_tensor(
                        ft[:, 0, :], ft[:, 0, :], ft[:, j, :], op=ALU.max
                    )
                nc.vector.reduce_max(val_h[:, t : t + 1], ft[:, 0, :], axis=X)

            for t in range(6):
                ft = fpool.tile([128, 8, 1024], f16, name="ft")
                eng = nc.sync if t % 2 == 0 else nc.scalar
                eng.dma_start(ft[:], xt_d[:, t, :])
                fold_reduce(ft, t)

            # slots 6,7: 4 chunked DMAs of 2048 cols; fold chunks 1..3 into
            # chunk 0's region as they arrive, then reduce chunk 0
            cht = {}
            for t in (6, 7):
                cht[t] = fpool.tile([128, 4, 2048], f16, name="ft")
            for ci in range(4):
                for t in (6, 7):
                    eng = nc.sync if t == 6 else nc.scalar
                    eng.dma_start(
                        cht[t][:, ci, :], xt_d[:, t, 2048 * ci : 2048 * (ci + 1)]
                    )
                    if ci > 0:
                        nc.vector.tensor_tensor(
                            cht[t][:, 0, :], cht[t][:, 0, :], cht[t][:, ci, :],
                            op=ALU.max,
                        )
            for t in (6, 7):
                nc.vector.reduce_max(val_h[:, t : t + 1], cht[t][:, 0, :], axis=X)
            nc.vector.tensor_copy(val32[:], val_h[:])

            # w3 is only needed for the tail GEMM. Its 48KB-contiguous
            # descriptors produce ~2us packets that win the per-packet
            # engine round-robin and starve the feat stream, so trickle it
            # in three 1MB chunks, each gated behind feat-stream progress
            # via a scalar-engine copy (the ACT FIFO delays the triggers).
            for g in range(3):
                nc.gpsimd.tensor_copy(scr2[:, g : g + 1], val_h[:, 2 * g + 1 : 2 * g + 2])
                nc.gpsimd.dma_start(w3t[:, g, :, :], w3_d[:, g, :, :])

            # --- PE warm-up burst, gated on slot 5's pooled value (lands a
            # few us before stream end) so HAM is warm for the tail matmuls ---
            gate = ps_s.tile([2, 1], f32, name="gate")
            nc.tensor.matmul(
                gate[:], val_h[:, 0:2], val_h[:, 5:6], start=True, stop=True
            )
            wps = ps_o.tile([128, 512], f32, name="pso")
            for _ in range(12):
                nc.tensor.matmul(
                    wps[0:16, :], w1[:, 0:16], w1[:, 0:512],
                    start=True, stop=True,
                )

            # --- LayerNorm per (group, col) on val32 [128, 8] ---
            red = ps_s.tile([2, 8], f32, name="red")
            nc.tensor.matmul(red[:], sel2T[:], val32[:], start=True, stop=True)
            nc.scalar.mul(mu2[:], red[:], 1.0 / FEAT)
            bc = ps_b.tile([128, 8], f32, name="bc")
            nc.tensor.matmul(bc[:], sel2[:], mu2[:], start=True, stop=True)
            nc.vector.tensor_tensor(zc[:], val32[:], bc[:], op=ALU.subtract)
            nc.vector.tensor_tensor(zsq[:], zc[:], zc[:], op=ALU.mult)
            red2 = ps_s.tile([2, 8], f32, name="red")
            nc.tensor.matmul(red2[:], sel2T[:], zsq[:], start=True, stop=True)
            nc.scalar.activation(
                std[:], red2[:], AF.Sqrt, bias=eps_t[:], scale=1.0 / FEAT
            )
            nc.vector.reciprocal(rstd[:], std[:])
            bc2 = ps_b.tile([128, 8], f32, name="bc")
            nc.tensor.matmul(bc2[:], sel2[:], rstd[:], start=True, stop=True)
            nc.vector.tensor_tensor(
                zn16[0:64, 0:8], zc[0:64, :], bc2[0:64, :], op=ALU.mult
            )
            nc.vector.tensor_tensor(
                zn16[64:128, 8:16], zc[64:128, :], bc2[64:128, :], op=ALU.mult
            )

            # --- proj (ln affine folded into wp/bp): lat[128m+p, s] ---
            for m in range(4):
                ps = ps_m.tile([128, 16], f32)
                nc.tensor.matmul(
                    ps[:], wp[:, 128 * m : 128 * (m + 1)], zn16[:],
                    start=True, stop=True,
                )
                nc.vector.tensor_scalar(
                    lat[:, 16 * m : 16 * (m + 1)], ps[:], bp[:, m : m + 1],
                    None, op0=ALU.add,
                )

            # --- h1 = relu(latent @ w1 + b1), transposed ---
            for n in range(2):
                ps = ps_m.tile([128, 16], f32)
                for k in range(4):
                    nc.tensor.matmul(
                        ps[:],
                        w1[:, (k * 2 + n) * 128 : (k * 2 + n + 1) * 128],
                        lat[:, 16 * k : 16 * (k + 1)],
                        start=(k == 0), stop=(k == 3),
                    )
                nc.vector.tensor_scalar(
                    h1[:, 16 * n : 16 * (n + 1)], ps[:], b1[:, n : n + 1],
                    0.0, op0=ALU.add, op1=ALU.max,
                )

            # --- h2 = relu(h1 @ w2 + b2), transposed ---
            for n in range(4):
                ps = ps_m.tile([128, 16], f32)
                for k in range(2):
                    nc.tensor.matmul(
                        ps[:],
                        w2[:, (k * 4 + n) * 128 : (k * 4 + n + 1) * 128],
                        h1[:, 16 * k : 16 * (k + 1)],
                        start=(k == 0), stop=(k == 1),
                    )
                nc.vector.tensor_scalar(
                    h2[:, 16 * n : 16 * (n + 1)], ps[:], b2[:, n : n + 1],
                    0.0, op0=ALU.add, op1=ALU.max,
                )

            # --- out = h2.T @ w3, col-tiled: 4 concurrent M=16 matmuls per
            # PSUM bank (tile_position col groups), copies split across
            # Vector/Scalar engines ---
            for g in range(3):
                psg = ps_o.tile([128, 512], f32, name="pso")
                for sub in range(4):
                    for k in range(4):
                        nc.tensor.matmul(
                            psg[32 * sub : 32 * sub + 16, :],
                            h2[:, 16 * k : 16 * (k + 1)],
                            w3t[:, g, k, 512 * sub : 512 * (sub + 1)],
                            start=(k == 0), stop=(k == 3),
                            tile_position=(0, 32 * sub),
                        )
                # one full-partition copy moves all 4 col-group quarters
                # (garbage rows included; the host slices them off)
                dst = out_sb[:, 512 * g : 512 * (g + 1)]
                if g % 2 == 0:
                    nc.vector.tensor_copy(dst, psg[:])
                else:
                    nc.scalar.copy(dst, psg[:])
                nc.sync.dma_start(
                    out_d[:, 512 * g : 512 * (g + 1)],
                    out_sb[:, 512 * g : 512 * (g + 1)],
                )

    nc.finalize()
    return nc


def _bf16(a):
    import ml_dtypes

    return np.ascontiguousarray(a.astype(ml_dtypes.bfloat16))


def pack_weights(ln_g, ln_b, proj_w, proj_b, w1, b1, w2, b2, w3, b3):
    c = np.ascontiguousarray
    wp = (ln_g[:, None] * proj_w).astype(np.float32)  # [64, 512]
    bpv = (ln_b.astype(np.float64) @ proj_w.astype(np.float64)).astype(np.float32) + proj_b
    return {
        "wp2": _bf16(np.vstack([wp, wp])),
        "bp": c(bpv.reshape(4, 128).T),
        "w1p": _bf16(
            w1.reshape(4, 128, 2, 128).transpose(1, 0, 2, 3).reshape(128, 1024)
        ),
        "b1p": c(b1.reshape(2, 128).T),
        "w2p": _bf16(
            w2.reshape(2, 128, 4, 128).transpose(1, 0, 2, 3).reshape(128, 1024)
        ),
        "b2p": c(b2.reshape(4, 128).T),
        "w3p": _bf16(w3.reshape(4, 128, 3, 2048).transpose(1, 2, 0, 3)),
        "selT": c(np.repeat(np.eye(2, dtype=np.float32), 64, axis=0)),
        "sel": c(np.repeat(np.eye(2, dtype=np.float32), 64, axis=1)),
    }


def pack_feat_core(feat16, feat32, bounds, c):
    xt = np.full((128, 8, S_PAD), F16MIN, np.float16)
    for sl in range(SEGS_PER_CORE):
        seg = c * SEGS_PER_CORE + sl
        a, b = bounds[seg], bounds[seg + 1]
        L = b - a
        if L > S_PAD:
            blk = np.concatenate(
                [
                    feat16[a : a + S_PAD - 1],
                    feat32[a + S_PAD - 1 : b].max(0, keepdims=True).astype(np.float16),
                ],
                0,
            )
            L = S_PAD
        else:
            blk = feat16[a:b]
        g, t = divmod(sl, 8)
        if L > 0:
            xt[g * 64 : (g + 1) * 64, t, :L] = blk.T
    return xt


def make_in_maps(inputs):
    feat32 = np.asarray(inputs["feat"], dtype=np.float32)
    feat16 = feat32.astype(np.float16)
    batch = np.asarray(inputs["batch"])
    wdict = pack_weights(
        *(np.asarray(inputs[k], dtype=np.float32) for k in
          ("ln_g", "ln_b", "proj_w", "proj_b", "w1", "b1", "w2", "b2", "w3", "b3"))
    )
    bounds = np.searchsorted(batch, np.arange(B + 1))
    return [
        {"xt": pack_feat_core(feat16, feat32, bounds, c), **wdict}
        for c in range(N_CORES)
    ]


def kernel(**inputs):
    from concourse.bass_utils import run_bass_kernel_spmd

    if "nc" not in _CACHE:
        _CACHE["nc"] = build_nc()
    nc = _CACHE["nc"]

    in_maps = make_in_maps(inputs)
    res = run_bass_kernel_spmd(nc, in_maps, list(range(N_CORES)))

    out = np.empty((B, OUT_F), np.float32)
    for c in range(N_CORES):
        r = np.asarray(res.results[c]["out"], dtype=np.float32)  # [128, 1536]
        # row 32*sub+s, col 512*g+n -> out[16c+s, 2048g+512sub+n]
        blk = r.reshape(4, 32, 3, 512)[:, :16]  # [sub, s, g, n]
        out[c * 16 : (c + 1) * 16] = blk.transpose(1, 2, 0, 3).reshape(16, OUT_F)
    out += np.asarray(inputs["b3"], dtype=np.float32)[None, :]
    return out.reshape(B, 2048, 3)
